# revision 1
# baseline (speedup 1.0000x reference)
"""Trainium2 Bass kernel for nn_CAGroup3DRoIHead (sparse conv + BN + ELU +
grid pooling + BN), 8-core SPMD.

Host: integer index computation (voxelize, unique, searchsorted hit list).
Device: all float work — gather sp_feats rows (windowed dma_gather), PE
transposes, per-offset matmuls vs W1, masked-BN stats, ELU, pooling conv
vs W2 (p-sharded across 8 cores, one-hot matmul aggregation over ROIs),
AllReduce, final BN.
"""
import numpy as np

G = 384
HALF = G // 2
SZ = G
SYZ = G * G
SXYZ = G * G * G
VOX = np.float32(0.08)
CK = 2
K = 5
GN = 7
EPS = 1e-5
C = 128
NCORES = 8
NV = 200000
N = 175616
P = GN ** 3          # 343
B_ROIS = 512
WIN = 32768
GCH = 2032           # max idx per dma_gather chunk
NQ = 4               # swdge queues

_cache = {}


def _wrap16(lst, pad_to):
    lst = np.asarray(lst, np.int64)
    n = len(lst)
    s = (pad_to + 15) // 16
    out = np.zeros((128, s), np.int16)
    padded = np.concatenate([lst, np.zeros(pad_to - n, np.int64)])
    for r in range(8):
        for p in range(16):
            row = padded[p::16]
            out[16 * r + p, :len(row)] = row
    return out


def _host_indices(sp_coords, grid_points):
    """Replicates reference.py's integer index math exactly."""
    sp_coords = np.asarray(sp_coords)
    grid_points = np.asarray(grid_points, np.float32)
    vox = np.clip(np.floor(grid_points[:, 1:4] / VOX).astype(np.int32),
                  -(HALF - 1), HALF - 1).astype(np.int64)
    pos = vox + HALF
    bidx = grid_points[:, 0].astype(np.int64)
    mc = bidx * SXYZ + pos[:, 0] * SYZ + pos[:, 1] * SZ + pos[:, 2]
    unq, unq_inv = np.unique(mc, return_inverse=True)
    Nq = len(unq)
    qb = unq // SXYZ
    qv = np.stack([unq % SXYZ // SYZ, unq % SYZ // SZ, unq % SZ], 1)

    svi = sp_coords[:, 1:4].astype(np.int64) // CK + HALF
    scode = sp_coords[:, 0].astype(np.int64) * SXYZ + svi[:, 0] * SYZ \
        + svi[:, 1] * SZ + svi[:, 2]
    order = np.argsort(scode, kind="stable")
    scodes = scode[order]

    kr = np.arange(-(K // 2), K // 2 + 1)
    offs = np.stack(np.meshgrid(kr, kr, kr, indexing="ij"), -1).reshape(-1, 3)
    hit_rows, hit_ks, hit_sp = [], [], []
    for k in range(K ** 3):
        tvi = qv + offs[k]
        inb = np.all((tvi >= 0) & (tvi < G), axis=1)
        code = qb * SXYZ + tvi[:, 0] * SYZ + tvi[:, 1] * SZ + tvi[:, 2]
        pp = np.clip(np.searchsorted(scodes, code), 0, NV - 1)
        hit = (scodes[pp] == code) & inb
        w = np.nonzero(hit)[0]
        hit_rows.append(w)
        hit_ks.append(np.full(len(w), k, np.int64))
        hit_sp.append(order[pp[w]])
    hit_rows = np.concatenate(hit_rows)   # already sorted by k (loop order)
    hit_ks = np.concatenate(hit_ks)
    hit_sp = np.concatenate(hit_sp)
    return Nq, unq_inv, hit_rows, hit_ks, hit_sp


def _build_layout(Nq, unq_inv, hit_rows, hit_ks, hit_sp):
    NHIT = len(hit_rows)
    ksizes = np.bincount(hit_ks, minlength=K ** 3)
    kstarts = np.concatenate([[0], np.cumsum(ksizes)])  # Y slot of hit j = j

    # F_win chunks: hits sorted by sp row, chunked by (window, <=GCH)
    order_sp = np.argsort(hit_sp, kind="stable")        # F slot -> hit id
    sp_sorted = hit_sp[order_sp]
    win_of = sp_sorted // WIN
    chunks = []
    dest = 0
    i = 0
    fwpos = np.empty(NHIT, np.int64)
    while i < NHIT:
        w = int(win_of[i])
        j = i
        while j < NHIT and j - i < GCH and win_of[j] == w:
            j += 1
        n = j - i
        npad = ((n + 127) // 128) * 128
        rel = sp_sorted[i:j] - w * WIN
        chunks.append((w, rel, dest, n, npad))
        fwpos[order_sp[i:j]] = dest + np.arange(n)
        dest += npad
        i = j
    NFW = dest

    # reorder: k-order slot j -> F_win col
    NYS = ((NHIT + 127) // 128) * 128
    reo = np.zeros(NYS, np.int64)
    reo[:NHIT] = fwpos

    # multi-hit rows -> ACCM
    rows_u, counts = np.unique(hit_rows, return_counts=True)
    multi_mask = counts >= 2
    multi_rows = rows_u[multi_mask]
    n_multi = len(multi_rows)
    ACCM_W = ((max(n_multi, 1) + 127) // 128) * 128
    # per row, list of its Y slots (hit ids), rows sorted by count desc
    ord_m = np.argsort(-counts[multi_mask], kind="stable")
    multi_rows = multi_rows[ord_m]
    mcounts = counts[multi_mask][ord_m]
    maxc = int(mcounts.max()) if n_multi else 1
    # hit ids grouped by row
    row_sorted = np.argsort(hit_rows, kind="stable")    # hit ids by row
    row_of = hit_rows[row_sorted]
    starts = np.searchsorted(row_of, multi_rows)
    rounds = []
    for j in range(maxc):
        sel = mcounts > j
        nsel = int(sel.sum())
        ridx = np.full(nsel, -1, np.int64)
        ridx = row_sorted[starts[sel] + j]              # hit id = Y slot
        rounds.append((ridx, nsel))

    # row -> xtab col ( ACCM | Y | zero )
    YBASE = ACCM_W
    ZCOL = ACCM_W + NYS
    XTW = ACCM_W + NYS + 128
    row2col = np.full(N, -1, np.int64)
    # single rows: their unique hit's Y slot
    single_rows = rows_u[~multi_mask]
    sstart = np.searchsorted(row_of, single_rows)
    row2col[single_rows] = YBASE + row_sorted[sstart]
    row2col[multi_rows] = np.arange(n_multi)
    return dict(NHIT=NHIT, ksizes=ksizes, kstarts=kstarts, chunks=chunks,
                NFW=NFW, NYS=NYS, reo=reo, n_multi=n_multi, ACCM_W=ACCM_W,
                rounds=rounds, YBASE=YBASE, ZCOL=ZCOL, XTW=XTW,
                row2col=row2col, Nq=Nq, unq_inv=unq_inv)


def _build_stage2(L):
    """Correction groups per p, sharded across cores. Returns per-core
    xg idx [43*128] and bv [43*128] plus static group count."""
    unq_inv = L["unq_inv"]
    row2col = L["row2col"]
    GP = 43                        # p-groups per core (core 7: 42 real + pad)
    cols = row2col[unq_inv]        # per grid slot
    nz = cols >= 0
    slot = np.arange(N)
    bb = slot // P
    pp = slot % P
    xg = np.full((NCORES, GP * 128), L["ZCOL"], np.int64)
    bv = np.full((NCORES, GP * 128), 600.0, np.float32)
    for c in range(NCORES):
        for gi in range(GP):
            p = c * GP + gi
            if p >= P:
                continue
            m = nz & (pp == p)
            cc = cols[m]
            bbv = bb[m]
            n = len(cc)
            assert n <= 128
            xg[c, gi * 128:gi * 128 + n] = cc
            bv[c, gi * 128:gi * 128 + n] = bbv
    return xg, bv, GP


def _compile(L, GP):
    import concourse.bass as bass
    import concourse.bacc as bacc
    import concourse.tile as tile
    from concourse import mybir
    from concourse.masks import make_identity

    f32 = mybir.dt.float32
    bf16 = mybir.dt.bfloat16
    i16 = mybir.dt.int16
    i32 = mybir.dt.int32
    AF = mybir.ActivationFunctionType
    OP = mybir.AluOpType
    AX = mybir.AxisListType

    chunks = L["chunks"]
    NCHK = len(chunks)
    SW = max((c[4] + 15) // 16 for c in chunks)
    NFW = L["NFW"]
    NYS = L["NYS"]
    NHIT = L["NHIT"]
    ACCM_W = L["ACCM_W"]
    XTW = L["XTW"]
    kstarts = L["kstarts"]
    rounds = L["rounds"]
    Nq = float(L["Nq"])
    NYCH = (NHIT + 511) // 512
    NR = len(rounds)

    nc = bacc.Bacc("TRN2", target_bir_lowering=False, debug=False,
                   num_devices=NCORES, num_swdge_queues=NQ)
    sp = nc.declare_dram_parameter("sp", [NV, C], f32, isOutput=False)
    w1 = nc.declare_dram_parameter("w1", [C, K ** 3 * C], f32, isOutput=False)
    w2 = nc.declare_dram_parameter("w2", [C, GP * C], f32, isOutput=False)
    g1 = nc.declare_dram_parameter("g1", [C, 1], f32, isOutput=False)
    b1 = nc.declare_dram_parameter("b1", [C, 1], f32, isOutput=False)
    g2 = nc.declare_dram_parameter("g2", [C, 1], f32, isOutput=False)
    b2 = nc.declare_dram_parameter("b2", [C, 1], f32, isOutput=False)
    widx = nc.declare_dram_parameter("widx", [128, NCHK * SW], i16, isOutput=False)
    reot = nc.declare_dram_parameter("reot", [128, NYS // 16], i16, isOutput=False)
    RW = sum(((r[1] + 127) // 128) * 128 for r in rounds) if rounds else 128
    rndt = nc.declare_dram_parameter("rndt", [128, RW // 16], i16, isOutput=False)
    xgt = nc.declare_dram_parameter("xgt", [128, GP * 128 // 16], i16, isOutput=False)
    bvt = nc.declare_dram_parameter("bvt", [128, GP], f32, isOutput=False)
    iot = nc.declare_dram_parameter("iot", [128, 512], f32, isOutput=False)
    out = nc.declare_dram_parameter("out", [B_ROIS, C], f32, isOutput=True)
    cc_in = nc.dram_tensor("cc_in", [C, B_ROIS], f32)
    cc_out = nc.dram_tensor("cc_out", [C, B_ROIS], f32)

    # st2 fused tile layout (f32 cols), reuses bufA slot after ft_k dies
    NXG = GP * 128
    o_xg = 0
    o_w2 = o_xg + NXG
    o_rt = o_w2 + GP * C
    o_sq = o_rt + ACCM_W
    o_io = max(o_sq + ACCM_W, o_rt + NXG)
    o_pl = o_io + 512
    o_pf = o_pl + 512
    o_ws = o_pf + 512
    ST2W = o_ws + C

    with tile.TileContext(nc) as tc:
        with (
            tc.tile_pool(name="sm", bufs=2) as sm,
            tc.tile_pool(name="big", bufs=1) as big,
            tc.tile_pool(name="pa", bufs=3, space="PSUM") as pa,
            tc.tile_pool(name="pb", bufs=2, space="PSUM") as pb,
            tc.tile_pool(name="pc", bufs=1, space="PSUM") as pc,
        ):
            ident = big.tile([128, 128], f32)
            make_identity(nc, ident[:])
            epst = big.tile([128, 1], f32)
            nc.vector.memset(epst[:], EPS)
            widx_t = big.tile([128, NCHK * SW], i16)
            nc.sync.dma_start(out=widx_t[:], in_=widx[:])
            reo_t = big.tile([128, NYS // 16], i16)
            nc.sync.dma_start(out=reo_t[:], in_=reot[:])
            rnd_t = big.tile([128, RW // 16], i16)
            nc.sync.dma_start(out=rnd_t[:], in_=rndt[:])
            xg_t = big.tile([128, GP * 128 // 16], i16)
            nc.sync.dma_start(out=xg_t[:], in_=xgt[:])
            bv_t = big.tile([128, GP], f32)
            nc.sync.dma_start(out=bv_t[:], in_=bvt[:])
            g1t = big.tile([128, 1], f32); nc.sync.dma_start(out=g1t[:], in_=g1[:])
            b1t = big.tile([128, 1], f32); nc.sync.dma_start(out=b1t[:], in_=b1[:])
            g2t = big.tile([128, 1], f32); nc.sync.dma_start(out=g2t[:], in_=g2[:])
            b2t = big.tile([128, 1], f32); nc.sync.dma_start(out=b2t[:], in_=b2[:])

            # ---------- stage 1: gather F ----------
            f_win = big.tile([128, max(NFW, ST2W)], f32, tag="bufA")
            for ci, (w, rel, d0, n, npad) in enumerate(chunks):
                nc.gpsimd.dma_gather(
                    out_ap=f_win[:, d0:d0 + npad].rearrange(
                        "p (b c) -> p b c", c=C),
                    in_ap=sp[w * WIN:min((w + 1) * WIN, NV), :],
                    idxs_ap=widx_t[:, ci * SW:ci * SW + npad // 16],
                    num_idxs=npad, num_idxs_reg=npad, elem_size=C,
                    single_packet=False, queue_num=ci % NQ,
                )

            # ---------- transposes ----------
            f_wint = big.tile([128, max(NFW, XTW)], f32, tag="bufB")
            ntile = NFW // 128
            for q0 in range(0, ntile, 4):
                qn = min(4, ntile - q0)
                tp = pa.tile([128, 512], f32, tag="a")
                for q in range(q0, q0 + qn):
                    nc.tensor.transpose(
                        out=tp[:, (q - q0) * 128:(q - q0 + 1) * 128],
                        in_=f_win[:, q * 128:(q + 1) * 128],
                        identity=ident[:])
                nc.scalar.activation(out=f_wint[:, q0 * 128:(q0 + qn) * 128],
                                     in_=tp[:, :qn * 128], func=AF.Copy)

            # ---------- reorder to k-order ----------
            ft_k = big.tile([128, max(NFW, ST2W)], f32, tag="bufA")
            nc.gpsimd.ap_gather(
                out_ap=ft_k[:, :NYS].rearrange("p (n u) -> p n u", u=1),
                in_ap=f_wint[:, :NFW].rearrange("p (n u) -> p n u", u=1),
                idxs_ap=reo_t[:], channels=128, num_elems=NFW, d=1,
                num_idxs=NYS)

            # ---------- matmuls vs W1 + stats ----------
            xtab = big.tile([128, max(NFW, XTW)], f32, tag="bufB")
            nc.vector.memset(xtab[:, ACCM_W + NYS:XTW], 0.0)
            sums = big.tile([128, NYCH], f32)
            sqs = big.tile([128, NYCH], f32)
            scr = sm.tile([128, 512], f32, tag="scr")
            KPH = 16
            w1ph = {}
            for ph in range((K ** 3 + KPH - 1) // KPH):
                k0, k1 = ph * KPH, min((ph + 1) * KPH, K ** 3)
                t = sm.tile([128, (k1 - k0) * C], f32, tag="w1c")
                nc.sync.dma_start(out=t[:], in_=w1[:, k0 * C:k1 * C])
                w1ph[ph] = t
            for ch in range(NYCH):
                c0, c1 = ch * 512, min(ch * 512 + 512, NHIT)
                ypt = pb.tile([128, 512], f32, tag="yt")
                for k in range(K ** 3):
                    a, b = int(kstarts[k]), int(kstarts[k + 1])
                    a2, b2_ = max(a, c0), min(b, c1)
                    if a2 >= b2_:
                        continue
                    t = w1ph[k // KPH]
                    ko = (k % KPH) * C
                    nc.tensor.matmul(out=ypt[:, a2 - c0:b2_ - c0],
                                     lhsT=t[:, ko:ko + C], rhs=ft_k[:, a2:b2_],
                                     start=True, stop=True)
                w = c1 - c0
                nc.scalar.activation(out=xtab[:, ACCM_W + c0:ACCM_W + c0 + w],
                                     in_=ypt[:, :w], func=AF.Copy)
                nc.vector.reduce_sum(out=sums[:, ch:ch + 1], in_=ypt[:, :w],
                                     axis=AX.X)
                nc.scalar.activation(out=scr[:, :w], in_=ypt[:, :w],
                                     func=AF.Square,
                                     accum_out=sqs[:, ch:ch + 1])

            # ---------- stage-2 fused tile (reuses bufA) ----------
            st2 = big.tile([128, max(NFW, ST2W)], f32, tag="bufA")

            # ---------- ACCM: multi-hit row sums (into xtab directly) ----------
            msq = big.tile([128, NR + 1], f32)
            roff = 0
            for j, (ridx, nsel) in enumerate(rounds):
                npad = ((nsel + 127) // 128) * 128
                if j == 0:
                    dst = xtab[:, :npad]
                else:
                    dst = st2[:, o_rt:o_rt + npad]
                nc.gpsimd.ap_gather(
                    out_ap=dst.rearrange("p (n u) -> p n u", u=1),
                    in_ap=xtab[:, ACCM_W:XTW].rearrange("p (n u) -> p n u", u=1),
                    idxs_ap=rnd_t[:, roff // 16:(roff + npad) // 16],
                    channels=128, num_elems=NYS + 128, d=1, num_idxs=npad)
                roff += npad
                nc.scalar.activation(out=st2[:, o_sq:o_sq + npad], in_=dst,
                                     func=AF.Square, accum_out=msq[:, j:j + 1])
                if j > 0:
                    nc.vector.tensor_tensor(out=xtab[:, :npad],
                                            in0=xtab[:, :npad],
                                            in1=st2[:, o_rt:o_rt + npad],
                                            op=OP.add)
            if ACCM_W > ((rounds[0][1] + 127) // 128) * 128:
                nc.vector.memset(
                    xtab[:, ((rounds[0][1] + 127) // 128) * 128:ACCM_W], 0.0)
            nc.scalar.activation(out=st2[:, o_sq:o_sq + ACCM_W],
                                 in_=xtab[:, :ACCM_W],
                                 func=AF.Square, accum_out=msq[:, NR:NR + 1])

            # ---------- stats finalize ----------
            st = big.tile([128, 4], f32)
            nc.vector.reduce_sum(out=st[:, 0:1], in_=sums[:], axis=AX.X)
            nc.vector.reduce_sum(out=st[:, 1:2], in_=sqs[:], axis=AX.X)
            nc.vector.reduce_sum(out=st[:, 2:3], in_=msq[:, :NR], axis=AX.X)
            nc.vector.tensor_tensor(out=st[:, 1:2], in0=st[:, 1:2],
                                    in1=st[:, 2:3], op=OP.subtract)
            nc.vector.tensor_tensor(out=st[:, 1:2], in0=st[:, 1:2],
                                    in1=msq[:, NR:NR + 1], op=OP.add)
            mean = big.tile([128, 1], f32)
            nc.vector.tensor_scalar_mul(out=mean[:], in0=st[:, 0:1],
                                        scalar1=1.0 / Nq)
            var = big.tile([128, 1], f32)
            nc.vector.tensor_scalar_mul(out=var[:], in0=st[:, 1:2],
                                        scalar1=1.0 / Nq)
            m2 = big.tile([128, 1], f32)
            nc.vector.tensor_tensor(out=m2[:], in0=mean[:], in1=mean[:],
                                    op=OP.mult)
            nc.vector.tensor_tensor(out=var[:], in0=var[:], in1=m2[:],
                                    op=OP.subtract)
            sd = big.tile([128, 1], f32)
            nc.scalar.activation(out=sd[:], in_=var[:], func=AF.Sqrt, bias=epst[:, :1])
            rs = big.tile([128, 1], f32)
            nc.vector.reciprocal(out=rs[:], in_=sd[:])
            rsg = big.tile([128, 1], f32)
            nc.vector.tensor_tensor(out=rsg[:], in0=rs[:], in1=g1t[:], op=OP.mult)
            shift = big.tile([128, 1], f32)
            nc.vector.tensor_tensor(out=shift[:], in0=mean[:], in1=rsg[:],
                                    op=OP.mult)
            nc.vector.tensor_tensor(out=shift[:], in0=b1t[:], in1=shift[:],
                                    op=OP.subtract)
            xz = big.tile([128, 1], f32)
            t1 = big.tile([128, 1], f32)
            nc.scalar.activation(out=xz[:], in_=shift[:], func=AF.Relu)
            nc.vector.tensor_scalar_min(out=t1[:], in0=shift[:], scalar1=0.0)
            nc.scalar.activation(out=t1[:], in_=t1[:], func=AF.Exp)
            nc.vector.tensor_tensor(out=xz[:], in0=xz[:], in1=t1[:], op=OP.add)
            nc.vector.tensor_scalar_add(out=xz[:], in0=xz[:], scalar1=-1.0)
            ccol = big.tile([128, 1], f32)
            nc.vector.tensor_scalar(out=ccol[:], in0=xz[:], scalar1=-1.0,
                                    scalar2=-1.0, op0=OP.mult, op1=OP.add)

            # ---------- stage 2 ----------
            nc.gpsimd.ap_gather(
                out_ap=st2[:, o_xg:o_xg + NXG].rearrange("p (n u) -> p n u", u=1),
                in_ap=xtab[:, :XTW].rearrange("p (n u) -> p n u", u=1),
                idxs_ap=xg_t[:], channels=128, num_elems=XTW, d=1,
                num_idxs=NXG)
            s_ = st2[:, o_xg:o_xg + NXG]
            r_ = st2[:, o_rt:o_rt + NXG]
            nc.vector.tensor_scalar(out=s_, in0=s_, scalar1=rsg[:, :1],
                                    scalar2=shift[:, :1], op0=OP.mult,
                                    op1=OP.add)
            nc.scalar.activation(out=r_, in_=s_, func=AF.Relu)
            nc.vector.tensor_scalar_min(out=s_, in0=s_, scalar1=0.0)
            nc.scalar.activation(out=s_, in_=s_, func=AF.Exp)
            nc.vector.tensor_tensor(out=s_, in0=s_, in1=r_, op=OP.add)
            nc.vector.tensor_scalar(out=s_, in0=s_, scalar1=ccol[:, :1],
                                    scalar2=None, op0=OP.add)

            nc.sync.dma_start(out=st2[:, o_w2:o_w2 + GP * C], in_=w2[:])
            wsc = st2[:, o_rt:o_rt + GP * C]
            nc.vector.tensor_copy(out=wsc, in_=st2[:, o_w2:o_w2 + GP * C])
            nfold = GP
            while nfold > 1:
                h = nfold // 2
                nc.vector.tensor_tensor(
                    out=st2[:, o_rt:o_rt + h * C],
                    in0=st2[:, o_rt:o_rt + h * C],
                    in1=st2[:, o_rt + (nfold - h) * C:o_rt + nfold * C],
                    op=OP.add)
                nfold -= h
            nc.vector.tensor_copy(out=st2[:, o_ws:o_ws + C],
                                  in_=st2[:, o_rt:o_rt + C])

            nc.sync.dma_start(out=st2[:, o_io:o_io + 512], in_=iot[:])
            pool_p = pc.tile([128, 512], f32, tag="pool")
            for q0 in range(0, GP, 4):
                qn = min(4, GP - q0)
                cp = pb.tile([128, 512], f32, tag="yt")
                for p in range(q0, q0 + qn):
                    nc.tensor.matmul(
                        out=cp[:, (p - q0) * 128:(p - q0 + 1) * 128],
                        lhsT=st2[:, o_xg + p * 128:o_xg + (p + 1) * 128],
                        rhs=st2[:, o_w2 + p * C:o_w2 + (p + 1) * C],
                        start=True, stop=True)
                cbf = sm.tile([128, 512], bf16, tag="cbf")
                nc.vector.tensor_copy(out=cbf[:, :qn * 128],
                                      in_=cp[:, :qn * 128])
                for p in range(q0, q0 + qn):
                    oh = sm.tile([128, 512], bf16, tag="oh")
                    nc.vector.tensor_tensor(
                        out=oh[:], in0=bv_t[:, p:p + 1].to_broadcast([128, 512]),
                        in1=st2[:, o_io:o_io + 512], op=OP.is_equal)
                    nc.tensor.matmul(out=pool_p[:],
                                     lhsT=cbf[:, (p - q0) * 128:(p - q0 + 1) * 128],
                                     rhs=oh[:], start=(p == 0),
                                     stop=(p == GP - 1))
            basep = pa.tile([128, 1], f32, tag="a")
            nc.tensor.matmul(out=basep[:], lhsT=st2[:, o_ws:o_ws + C],
                             rhs=xz[:, :1], start=True, stop=True)
            base = big.tile([128, 1], f32)
            nc.vector.tensor_copy(out=base[:], in_=basep[:])
            nc.vector.tensor_copy(out=st2[:, o_pl:o_pl + 512], in_=pool_p[:])
            nc.vector.tensor_scalar(out=st2[:, o_pl:o_pl + 512],
                                    in0=st2[:, o_pl:o_pl + 512],
                                    scalar1=base[:, :1], scalar2=None,
                                    op0=OP.add)

            # ---------- AllReduce + final BN ----------
            nc.sync.dma_start(out=cc_in[:], in_=st2[:, o_pl:o_pl + 512])
            nc.gpsimd.collective_compute(
                "AllReduce", OP.add, replica_groups=[list(range(NCORES))],
                ins=[cc_in[:]], outs=[cc_out[:]])
            pf = st2[:, o_pf:o_pf + 512]
            nc.sync.dma_start(out=pf, in_=cc_out[:])
            mn2 = big.tile([128, 1], f32)
            nc.vector.reduce_sum(out=mn2[:], in_=pf, axis=AX.X)
            nc.vector.tensor_scalar_mul(out=mn2[:], in0=mn2[:],
                                        scalar1=1.0 / B_ROIS)
            sq2 = big.tile([128, 1], f32)
            nc.scalar.activation(out=scr[:], in_=pf, func=AF.Square,
                                 accum_out=sq2[:])
            nc.vector.tensor_scalar_mul(out=sq2[:], in0=sq2[:],
                                        scalar1=1.0 / B_ROIS)
            m22 = big.tile([128, 1], f32)
            nc.vector.tensor_tensor(out=m22[:], in0=mn2[:], in1=mn2[:],
                                    op=OP.mult)
            nc.vector.tensor_tensor(out=sq2[:], in0=sq2[:], in1=m22[:],
                                    op=OP.subtract)
            sd2 = big.tile([128, 1], f32)
            nc.scalar.activation(out=sd2[:], in_=sq2[:], func=AF.Sqrt, bias=epst[:, :1])
            rs2 = big.tile([128, 1], f32)
            nc.vector.reciprocal(out=rs2[:], in_=sd2[:])
            rsg2 = big.tile([128, 1], f32)
            nc.vector.tensor_tensor(out=rsg2[:], in0=rs2[:], in1=g2t[:],
                                    op=OP.mult)
            sh2 = big.tile([128, 1], f32)
            nc.vector.tensor_tensor(out=sh2[:], in0=mn2[:], in1=rsg2[:],
                                    op=OP.mult)
            nc.vector.tensor_tensor(out=sh2[:], in0=b2t[:], in1=sh2[:],
                                    op=OP.subtract)
            nc.vector.tensor_scalar(out=pf, in0=pf, scalar1=rsg2[:, :1],
                                    scalar2=sh2[:, :1], op0=OP.mult,
                                    op1=OP.add)
            for t in range(4):
                op_ = pa.tile([128, 128], f32, tag="a")
                nc.tensor.transpose(out=op_[:],
                                    in_=st2[:, o_pf + t * 128:o_pf + (t + 1) * 128],
                                    identity=ident[:])
                os_ = sm.tile([128, 128], f32, tag="os")
                nc.vector.tensor_copy(out=os_[:], in_=op_[:])
                nc.sync.dma_start(out=out[t * 128:(t + 1) * 128, :], in_=os_[:])

    nc.compile()
    return nc


def kernel(**inputs):
    sp_coords = np.asarray(inputs["sp_coords"])
    sp_feats = np.asarray(inputs["sp_feats"], np.float32)
    grid_points = np.asarray(inputs["grid_points"], np.float32)
    W1 = np.asarray(inputs["W1"], np.float32)
    gamma1 = np.asarray(inputs["gamma1"], np.float32)
    beta1 = np.asarray(inputs["beta1"], np.float32)
    W2 = np.asarray(inputs["W2"], np.float32)
    gamma2 = np.asarray(inputs["gamma2"], np.float32)
    beta2 = np.asarray(inputs["beta2"], np.float32)

    Nq, unq_inv, hit_rows, hit_ks, hit_sp = _host_indices(sp_coords, grid_points)
    L = _build_layout(Nq, unq_inv, hit_rows, hit_ks, hit_sp)
    xg, bv, GP = _build_stage2(L)

    key = (L["NHIT"], tuple(L["ksizes"].tolist()), L["NFW"], L["n_multi"],
           tuple(r[1] for r in L["rounds"]), L["Nq"])
    if key not in _cache:
        _cache.clear()
        _cache[key] = _compile(L, GP)
    nc = _cache[key]

    chunks = L["chunks"]
    SW = max((c[4] + 15) // 16 for c in chunks)
    widx_np = np.zeros((128, len(chunks) * SW), np.int16)
    for ci, (w, rel, d0, n, npad) in enumerate(chunks):
        a = _wrap16(np.concatenate([rel, np.zeros(npad - n, np.int64)]), npad)
        widx_np[:, ci * SW:ci * SW + a.shape[1]] = a
    reo_np = _wrap16(L["reo"], L["NYS"])
    # rounds idx (relative to Y region start; pad -> zero col = NYS..)
    rparts = []
    for (ridx, nsel) in L["rounds"]:
        npad = ((nsel + 127) // 128) * 128
        rparts.append(np.concatenate(
            [ridx, np.full(npad - nsel, L["NYS"], np.int64)]))
    rnd_all = np.concatenate(rparts) if rparts else np.zeros(128, np.int64)
    rnd_np = _wrap16(rnd_all, len(rnd_all))

    W1t = np.ascontiguousarray(
        W1.transpose(1, 0, 2).reshape(C, K ** 3 * C))
    base_in = {
        "sp": sp_feats, "w1": W1t,
        "g1": gamma1.reshape(C, 1), "b1": beta1.reshape(C, 1),
        "g2": gamma2.reshape(C, 1), "b2": beta2.reshape(C, 1),
        "widx": widx_np, "reot": reo_np, "rndt": rnd_np,
        "iot": np.broadcast_to(np.arange(512, dtype=np.float32), (128, 512)).copy(),
    }
    in_maps = []
    for c in range(NCORES):
        m = dict(base_in)
        w2l = np.zeros((GP, C, C), np.float32)
        p0 = c * GP
        nreal = max(0, min(GP, 343 - p0))
        w2l[:nreal] = W2[p0:p0 + nreal]
        m["w2"] = np.ascontiguousarray(
            w2l.transpose(1, 0, 2).reshape(C, GP * C))
        m["xgt"] = _wrap16(xg[c], GP * 128)
        bvw = np.zeros((128, GP), np.float32)
        for g in range(GP):
            bvw[:, g] = bv[c, g * 128:(g + 1) * 128]
        m["bvt"] = bvw
        in_maps.append(m)

    import os
    from concourse.bass_utils import run_bass_kernel_spmd
    trace = os.environ.get("KERNEL_TRACE", "0") == "1"
    if trace:
        try:
            import ntff_hook
            ntff_hook.install()
        except Exception:
            trace = False
    res = run_bass_kernel_spmd(nc, in_maps, list(range(NCORES)), trace=trace)
    if trace and res.exec_time_ns:
        print("HW exec time: %d ns" % res.exec_time_ns)
    return np.asarray(res.results[0]["out"], np.float32)



# revision 3
# speedup vs baseline: 2.9683x; 2.9683x over previous
"""Trainium2 Bass kernel for nn_CAGroup3DRoIHead (sparse conv + BN + ELU +
grid pooling + BN), 8-core SPMD.

Sharding: stage 1 (sparse conv) and stage 2 (grid pooling) are both
sharded by pooling cell p (43 cells per core); each core only processes
the unique voxels its cells reference.  Host does all integer index
math and pre-gathers sp_feats rows into a k-major padded table per core
(pure data movement); device does all float compute: 125 matmuls vs W1
(bf16), multi-hit row sums, global-BN stats partials + tiny AllReduce,
BN+ELU, pooling conv vs W2 (bf16, one-hot matmul aggregation over ROIs),
pooled AllReduce, final BN.
"""
import numpy as np
import ml_dtypes

G = 384
HALF = G // 2
SZ = G
SYZ = G * G
SXYZ = G * G * G
VOX = np.float32(0.08)
CK = 2
K = 5
K3 = K ** 3
GN = 7
EPS = 1e-5
C = 128
NCORES = 8
NV = 200000
N = 175616
P = GN ** 3          # 343
B_ROIS = 512
GP = 43              # p-cells per core (core 7: 42 real + 1 pad)
SLOT = 128           # padded slots per p-cell

BF16 = ml_dtypes.bfloat16
_cache = {}


def _pad(n, m):
    return ((int(n) + m - 1) // m) * m


def _wrap16(lst, pad_to):
    lst = np.asarray(lst, np.int64)
    n = len(lst)
    s = (pad_to + 15) // 16
    out = np.zeros((128, s), np.int16)
    padded = np.concatenate([lst, np.zeros(pad_to - n, np.int64)])
    for r in range(8):
        for p in range(16):
            row = padded[p::16]
            out[16 * r + p, :len(row)] = row
    return out


def _host_indices(sp_coords, grid_points):
    """Replicates reference.py's integer index math exactly."""
    sp_coords = np.asarray(sp_coords)
    grid_points = np.asarray(grid_points, np.float32)
    vox = np.clip(np.floor(grid_points[:, 1:4] / VOX).astype(np.int32),
                  -(HALF - 1), HALF - 1).astype(np.int64)
    pos = vox + HALF
    bidx = grid_points[:, 0].astype(np.int64)
    mc = bidx * SXYZ + pos[:, 0] * SYZ + pos[:, 1] * SZ + pos[:, 2]
    unq, unq_inv = np.unique(mc, return_inverse=True)
    Nq = len(unq)
    qb = unq // SXYZ
    qv = np.stack([unq % SXYZ // SYZ, unq % SYZ // SZ, unq % SZ], 1)

    svi = sp_coords[:, 1:4].astype(np.int64) // CK + HALF
    scode = sp_coords[:, 0].astype(np.int64) * SXYZ + svi[:, 0] * SYZ \
        + svi[:, 1] * SZ + svi[:, 2]
    order = np.argsort(scode, kind="stable")
    scodes = scode[order]

    kr = np.arange(-(K // 2), K // 2 + 1)
    offs = np.stack(np.meshgrid(kr, kr, kr, indexing="ij"), -1).reshape(-1, 3)
    hit_rows, hit_ks, hit_sp = [], [], []
    for k in range(K3):
        tvi = qv + offs[k]
        inb = np.all((tvi >= 0) & (tvi < G), axis=1)
        code = qb * SXYZ + tvi[:, 0] * SYZ + tvi[:, 1] * SZ + tvi[:, 2]
        pp = np.clip(np.searchsorted(scodes, code), 0, NV - 1)
        hit = (scodes[pp] == code) & inb
        w = np.nonzero(hit)[0]
        hit_rows.append(w)
        hit_ks.append(np.full(len(w), k, np.int64))
        hit_sp.append(order[pp[w]])
    hit_rows = np.concatenate(hit_rows)   # voxel slot of hit, k-major order
    hit_ks = np.concatenate(hit_ks)
    hit_sp = np.concatenate(hit_sp)
    return Nq, unq_inv, hit_rows, hit_ks, hit_sp


def _shard(Nq, unq_inv, hit_rows, hit_ks, hit_sp):
    """Partition work by pooling cell p; build per-core index tables."""
    pp = np.arange(N) % P
    bb = np.arange(N) // P
    core_slot = np.minimum(pp // GP, NCORES - 1)
    order_v = np.argsort(hit_rows, kind="stable")
    row_of = hit_rows[order_v]
    has_hit = np.zeros(Nq, bool)
    has_hit[hit_rows] = True
    owner = np.full(Nq, NCORES, np.int64)
    for c in range(NCORES - 1, -1, -1):
        owner[np.unique(unq_inv[core_slot == c])] = c

    # pass 1: per-core raw structure
    raw = []
    for c in range(NCORES):
        slots_c = np.nonzero((core_slot == c) & has_hit[unq_inv])[0]
        vox_c = np.unique(unq_inv[slots_c])          # sorted, all have hits
        a = np.searchsorted(row_of, vox_c, "left")
        b = np.searchsorted(row_of, vox_c, "right")
        cnt = b - a
        all_h = np.concatenate([order_v[x:y] for x, y in zip(a, b)]) \
            if len(vox_c) else np.zeros(0, np.int64)
        raw.append((slots_c, vox_c, a, cnt, all_h))

    # shared padded sizes (max over cores)
    KW = 4
    maxcnt = 2
    max_multi = 1
    max_owned = 1
    for c in range(NCORES):
        slots_c, vox_c, a, cnt, all_h = raw[c]
        kc = np.bincount(hit_ks[all_h], minlength=K3)
        KW = max(KW, _pad(kc.max(), 4))
        maxcnt = max(maxcnt, int(cnt.max()))
        max_multi = max(max_multi, int((cnt >= 2).sum()))
        max_owned = max(max_owned, int((owner[vox_c] == c).sum()))
    NYW = K3 * KW
    MW = _pad(max_multi, 16)
    NRND = maxcnt - 1                                # add-rounds j=1..NRND
    RWS = []
    for j in range(1, maxcnt):
        m = max(max(int((r[3] > j).sum()) for r in raw), 1)
        RWS.append(_pad(m, 4))
    GRW = _pad(sum(RWS), 16)
    SW = _pad(max_owned, 16)
    YB = MW + GRW                                    # Y region offset in xtab
    ZC = YB + NYW                                    # zero column (abs)
    XTW = ZC + 128
    XGW = GP * SLOT
    GW = _pad(SW + XGW, 16)

    cores = []
    for c in range(NCORES):
        slots_c, vox_c, a, cnt, all_h = raw[c]
        nv = len(vox_c)
        ks_h = hit_ks[all_h]
        sp_h = hit_sp[all_h]
        ord_k = np.argsort(ks_h, kind="stable")
        ks_sorted = ks_h[ord_k]
        kcounts = np.bincount(ks_sorted, minlength=K3)
        kstart = np.concatenate([[0], np.cumsum(kcounts)])[:-1]
        rank = np.arange(len(ks_sorted)) - kstart[ks_sorted]
        col_of_hit = np.empty(len(all_h), np.int64)
        col_of_hit[ord_k] = ks_sorted * KW + rank
        starts = np.concatenate([[0], np.cumsum(cnt)])
        first_col = col_of_hit[starts[:-1]]

        multi_mask = cnt >= 2
        m_idx = np.nonzero(multi_mask)[0]
        m_ord = np.argsort(-cnt[multi_mask], kind="stable")
        multi_vloc = m_idx[m_ord]                    # count-desc local vox ids
        n_multi = len(multi_vloc)
        mcnt = cnt[multi_vloc]

        acc_abs = np.where(multi_mask, 0, YB + first_col)
        acc_abs[multi_vloc] = np.arange(n_multi)

        # G12: [round0 (MW) | rounds j=1.. (GRW)], idx relative to Y+Z region
        g12 = np.full(MW + GRW, NYW, np.int64)       # pad -> zero col
        g12[:n_multi] = first_col[multi_vloc]
        off = MW
        for j in range(1, maxcnt):
            sel = multi_vloc[mcnt > j]               # prefix (count-desc)
            nj = len(sel)
            g12[off:off + nj] = col_of_hit[starts[sel] + j]
            off += RWS[j - 1]

        # G34: [stats gather (SW) | stage-2 xg (XGW)], idx absolute in xtab
        own = owner[vox_c] == c
        sg = np.full(GW, ZC, np.int64)
        oc = acc_abs[own]
        sg[:len(oc)] = oc
        bv = np.full(XGW, 600.0, np.float32)
        vox_of_slot = unq_inv[slots_c]
        p_of_slot = pp[slots_c]
        for lp in range(GP):
            p = c * GP + lp
            if p >= P:
                continue
            m = p_of_slot == p
            sl = slots_c[m]
            n = len(sl)
            assert n <= SLOT
            vl = np.searchsorted(vox_c, vox_of_slot[m])
            sg[SW + lp * SLOT: SW + lp * SLOT + n] = acc_abs[vl]
            bv[lp * SLOT: lp * SLOT + n] = bb[sl]
        cores.append(dict(fcols=col_of_hit, frows=sp_h, g12=g12, sg=sg,
                          bv=bv, n_multi=n_multi))
    sizes = dict(KW=KW, NYW=NYW, MW=MW, NRND=NRND, RWS=tuple(RWS), GRW=GRW,
                 SW=SW, YB=YB, ZC=ZC, XTW=XTW, XGW=XGW, GW=GW, Nq=Nq)
    return cores, sizes


def _compile(S):
    import concourse.bass as bass
    import concourse.bacc as bacc
    import concourse.tile as tile
    from concourse import mybir
    from concourse.masks import make_identity

    f32 = mybir.dt.float32
    bf16 = mybir.dt.bfloat16
    i16 = mybir.dt.int16
    AF = mybir.ActivationFunctionType
    OP = mybir.AluOpType
    AX = mybir.AxisListType

    KW, NYW, MW, NRND, RWS = S["KW"], S["NYW"], S["MW"], S["NRND"], S["RWS"]
    GRW, SW, YB, ZC, XTW = S["GRW"], S["SW"], S["YB"], S["ZC"], S["XTW"]
    XGW, GW, Nq = S["XGW"], S["GW"], float(S["Nq"])

    nc = bacc.Bacc("TRN2", target_bir_lowering=False, debug=False,
                   num_devices=NCORES, num_swdge_queues=1)
    f_p = nc.declare_dram_parameter("f", [C, NYW], bf16, isOutput=False)
    w1 = nc.declare_dram_parameter("w1", [C, K3 * C], bf16, isOutput=False)
    w2 = nc.declare_dram_parameter("w2", [C, GP * C], bf16, isOutput=False)
    ws = nc.declare_dram_parameter("ws", [C, C], f32, isOutput=False)
    g1 = nc.declare_dram_parameter("g1", [C, 1], f32, isOutput=False)
    b1 = nc.declare_dram_parameter("b1", [C, 1], f32, isOutput=False)
    g2 = nc.declare_dram_parameter("g2", [C, 1], f32, isOutput=False)
    b2 = nc.declare_dram_parameter("b2", [C, 1], f32, isOutput=False)
    g12t = nc.declare_dram_parameter("g12t", [128, (MW + GRW) // 16], i16,
                                     isOutput=False)
    g34t = nc.declare_dram_parameter("g34t", [128, GW // 16], i16,
                                     isOutput=False)
    bvt = nc.declare_dram_parameter("bvt", [128, GP], f32, isOutput=False)
    iot = nc.declare_dram_parameter("iot", [128, 512], f32, isOutput=False)
    out = nc.declare_dram_parameter("out", [B_ROIS, C], f32, isOutput=True)
    cc1i = nc.dram_tensor("cc1i", [C, 2], f32)
    cc1o = nc.dram_tensor("cc1o", [C, 2], f32)
    cc2i = nc.dram_tensor("cc2i", [C, B_ROIS], f32)
    cc2o = nc.dram_tensor("cc2o", [C, B_ROIS], f32)

    NCH = (NYW + 511) // 512

    with tile.TileContext(nc) as tc:
        with (
            tc.tile_pool(name="sm", bufs=2) as sm,
            tc.tile_pool(name="big", bufs=1) as big,
            tc.tile_pool(name="pa", bufs=2, space="PSUM") as pa,
            tc.tile_pool(name="pb", bufs=2, space="PSUM") as pb,
            tc.tile_pool(name="pc", bufs=1, space="PSUM") as pc,
        ):
            ident = big.tile([128, 128], f32)
            make_identity(nc, ident[:])
            epst = big.tile([128, 1], f32)
            nc.vector.memset(epst[:], EPS)
            g12_t = big.tile([128, (MW + GRW) // 16], i16)
            nc.sync.dma_start(out=g12_t[:], in_=g12t[:])
            g34_t = big.tile([128, GW // 16], i16)
            nc.sync.dma_start(out=g34_t[:], in_=g34t[:])
            bv_t = big.tile([128, GP], f32)
            nc.sync.dma_start(out=bv_t[:], in_=bvt[:])
            io_t = big.tile([128, 512], f32)
            nc.sync.dma_start(out=io_t[:], in_=iot[:])
            g1t = big.tile([128, 1], f32); nc.sync.dma_start(out=g1t[:], in_=g1[:])
            b1t = big.tile([128, 1], f32); nc.sync.dma_start(out=b1t[:], in_=b1[:])
            g2t = big.tile([128, 1], f32); nc.sync.dma_start(out=g2t[:], in_=g2[:])
            b2t = big.tile([128, 1], f32); nc.sync.dma_start(out=b2t[:], in_=b2[:])
            wst = big.tile([128, C], f32)
            nc.sync.dma_start(out=wst[:], in_=ws[:])

            ft = big.tile([128, NYW], bf16)
            nc.sync.dma_start(out=ft[:], in_=f_p[:])
            w1t = big.tile([128, K3 * C], bf16)
            HW1 = (K3 // 2) * C
            nc.sync.dma_start(out=w1t[:, :HW1], in_=w1[:, :HW1])
            nc.sync.dma_start(out=w1t[:, HW1:], in_=w1[:, HW1:K3 * C])
            w2t = big.tile([128, GP * C], bf16)
            nc.sync.dma_start(out=w2t[:], in_=w2[:])

            # one-hot ROI matrices per p-cell (DVE, overlaps the loads)
            oh = big.tile([128, GP * 512], bf16)
            for lp in range(GP):
                nc.vector.tensor_tensor(
                    out=oh[:, lp * 512:(lp + 1) * 512],
                    in0=bv_t[:, lp:lp + 1].to_broadcast([128, 512]),
                    in1=io_t[:], op=OP.is_equal)

            # ---------- stage 1: 125 matmuls vs W1 ----------
            xtab = big.tile([128, XTW], f32)
            nc.vector.memset(xtab[:, ZC:XTW], 0.0)
            for ch in range(NCH):
                c0, c1 = ch * 512, min(ch * 512 + 512, NYW)
                yp = pa.tile([128, 512], f32, tag="yp")
                for k in range(c0 // KW, (c1 + KW - 1) // KW):
                    a = max(k * KW, c0)
                    b = min((k + 1) * KW, c1)
                    if a >= b:
                        continue
                    nc.tensor.matmul(out=yp[:, a - c0:b - c0],
                                     lhsT=w1t[:, k * C:(k + 1) * C],
                                     rhs=ft[:, a:b], start=True, stop=True)
                nc.scalar.activation(out=xtab[:, YB + c0:YB + c1],
                                     in_=yp[:, :c1 - c0], func=AF.Copy)

            # ---------- multi-hit voxel sums ----------
            nc.gpsimd.ap_gather(
                out_ap=xtab[:, 0:MW + GRW].rearrange("p (n u) -> p n u", u=1),
                in_ap=xtab[:, YB:XTW].rearrange("p (n u) -> p n u", u=1),
                idxs_ap=g12_t[:], channels=128, num_elems=NYW + 128, d=1,
                num_idxs=MW + GRW)
            off = MW
            for j in range(NRND):
                rw = RWS[j]
                nc.vector.tensor_tensor(out=xtab[:, :rw], in0=xtab[:, :rw],
                                        in1=xtab[:, off:off + rw], op=OP.add)
                off += rw

            # ---------- gather stats cols + stage-2 cols ----------
            st2 = big.tile([128, GW], f32)
            nc.gpsimd.ap_gather(
                out_ap=st2[:, 0:GW].rearrange("p (n u) -> p n u", u=1),
                in_ap=xtab[:, 0:XTW].rearrange("p (n u) -> p n u", u=1),
                idxs_ap=g34_t[:], channels=128, num_elems=XTW, d=1,
                num_idxs=GW)

            # ---------- BN1 stats partials + AllReduce ----------
            scr = big.tile([128, SW], f32)
            st = big.tile([128, 2], f32)
            nc.vector.reduce_sum(out=st[:, 0:1], in_=st2[:, :SW], axis=AX.X)
            nc.scalar.activation(out=scr[:], in_=st2[:, :SW], func=AF.Square,
                                 accum_out=st[:, 1:2])
            nc.sync.dma_start(out=cc1i[:], in_=st[:])
            nc.gpsimd.collective_compute(
                "AllReduce", OP.add, replica_groups=[list(range(NCORES))],
                ins=[cc1i[:]], outs=[cc1o[:]])
            stg = big.tile([128, 2], f32)
            nc.sync.dma_start(out=stg[:], in_=cc1o[:])

            # ---------- BN1 constants ----------
            mean = big.tile([128, 1], f32)
            nc.vector.tensor_scalar_mul(out=mean[:], in0=stg[:, 0:1],
                                        scalar1=1.0 / Nq)
            var = big.tile([128, 1], f32)
            nc.vector.tensor_scalar_mul(out=var[:], in0=stg[:, 1:2],
                                        scalar1=1.0 / Nq)
            m2 = big.tile([128, 1], f32)
            nc.vector.tensor_tensor(out=m2[:], in0=mean[:], in1=mean[:],
                                    op=OP.mult)
            nc.vector.tensor_tensor(out=var[:], in0=var[:], in1=m2[:],
                                    op=OP.subtract)
            sd = big.tile([128, 1], f32)
            nc.scalar.activation(out=sd[:], in_=var[:], func=AF.Sqrt,
                                 bias=epst[:, :1])
            rs = big.tile([128, 1], f32)
            nc.vector.reciprocal(out=rs[:], in_=sd[:])
            rsg = big.tile([128, 1], f32)
            nc.vector.tensor_tensor(out=rsg[:], in0=rs[:], in1=g1t[:],
                                    op=OP.mult)
            shift = big.tile([128, 1], f32)
            nc.vector.tensor_tensor(out=shift[:], in0=mean[:], in1=rsg[:],
                                    op=OP.mult)
            nc.vector.tensor_tensor(out=shift[:], in0=b1t[:], in1=shift[:],
                                    op=OP.subtract)
            xz = big.tile([128, 1], f32)
            t1 = big.tile([128, 1], f32)
            nc.scalar.activation(out=xz[:], in_=shift[:], func=AF.Relu)
            nc.vector.tensor_scalar_min(out=t1[:], in0=shift[:], scalar1=0.0)
            nc.scalar.activation(out=t1[:], in_=t1[:], func=AF.Exp)
            nc.vector.tensor_tensor(out=xz[:], in0=xz[:], in1=t1[:], op=OP.add)
            nc.vector.tensor_scalar_add(out=xz[:], in0=xz[:], scalar1=-1.0)
            ccol = big.tile([128, 1], f32)
            nc.vector.tensor_scalar(out=ccol[:], in0=xz[:], scalar1=-1.0,
                                    scalar2=-1.0, op0=OP.mult, op1=OP.add)

            # ---------- BN + ELU on stage-2 cols, minus xz ----------
            sx = st2[:, SW:SW + XGW]
            r_ = big.tile([128, XGW], f32)
            nc.vector.tensor_scalar(out=sx, in0=sx, scalar1=rsg[:, :1],
                                    scalar2=shift[:, :1], op0=OP.mult,
                                    op1=OP.add)
            nc.scalar.activation(out=r_[:], in_=sx, func=AF.Relu)
            nc.vector.tensor_scalar_min(out=sx, in0=sx, scalar1=0.0)
            nc.scalar.activation(out=sx, in_=sx, func=AF.Exp)
            nc.vector.tensor_tensor(out=sx, in0=sx, in1=r_[:], op=OP.add)
            nc.vector.tensor_scalar(out=sx, in0=sx, scalar1=ccol[:, :1],
                                    scalar2=None, op0=OP.add)
            sxb = big.tile([128, XGW], bf16)
            nc.vector.tensor_copy(out=sxb[:], in_=sx)

            # ---------- pooling conv: corrections + one-hot aggregation ----
            pool_p = pc.tile([128, 512], f32, tag="pool")
            for q0 in range(0, GP, 4):
                qn = min(4, GP - q0)
                cp = pb.tile([128, 512], f32, tag="cp")
                for lp in range(q0, q0 + qn):
                    nc.tensor.matmul(
                        out=cp[:, (lp - q0) * 128:(lp - q0 + 1) * 128],
                        lhsT=sxb[:, lp * SLOT:(lp + 1) * SLOT],
                        rhs=w2t[:, lp * C:(lp + 1) * C],
                        start=True, stop=True)
                cbf = sm.tile([128, 512], bf16, tag="cbf")
                nc.vector.tensor_copy(out=cbf[:, :qn * 128],
                                      in_=cp[:, :qn * 128])
                for lp in range(q0, q0 + qn):
                    nc.tensor.matmul(
                        out=pool_p[:],
                        lhsT=cbf[:, (lp - q0) * 128:(lp - q0 + 1) * 128],
                        rhs=oh[:, lp * 512:(lp + 1) * 512],
                        start=(lp == 0), stop=(lp == GP - 1))
            basep = pa.tile([128, 1], f32, tag="yp")
            nc.tensor.matmul(out=basep[:], lhsT=wst[:], rhs=xz[:, :1],
                             start=True, stop=True)
            base = big.tile([128, 1], f32)
            nc.vector.tensor_copy(out=base[:], in_=basep[:])
            pl = big.tile([128, 512], f32)
            nc.vector.tensor_copy(out=pl[:], in_=pool_p[:])
            nc.vector.tensor_scalar(out=pl[:], in0=pl[:], scalar1=base[:, :1],
                                    scalar2=None, op0=OP.add)

            # ---------- pooled AllReduce + final BN ----------
            nc.sync.dma_start(out=cc2i[:], in_=pl[:])
            nc.gpsimd.collective_compute(
                "AllReduce", OP.add, replica_groups=[list(range(NCORES))],
                ins=[cc2i[:]], outs=[cc2o[:]])
            pf = big.tile([128, 512], f32)
            nc.sync.dma_start(out=pf[:], in_=cc2o[:])
            mn2 = big.tile([128, 1], f32)
            nc.vector.reduce_sum(out=mn2[:], in_=pf[:], axis=AX.X)
            nc.vector.tensor_scalar_mul(out=mn2[:], in0=mn2[:],
                                        scalar1=1.0 / B_ROIS)
            sq2 = big.tile([128, 1], f32)
            scr2 = big.tile([128, 512], f32)
            nc.scalar.activation(out=scr2[:], in_=pf[:], func=AF.Square,
                                 accum_out=sq2[:])
            nc.vector.tensor_scalar_mul(out=sq2[:], in0=sq2[:],
                                        scalar1=1.0 / B_ROIS)
            m22 = big.tile([128, 1], f32)
            nc.vector.tensor_tensor(out=m22[:], in0=mn2[:], in1=mn2[:],
                                    op=OP.mult)
            nc.vector.tensor_tensor(out=sq2[:], in0=sq2[:], in1=m22[:],
                                    op=OP.subtract)
            sd2 = big.tile([128, 1], f32)
            nc.scalar.activation(out=sd2[:], in_=sq2[:], func=AF.Sqrt,
                                 bias=epst[:, :1])
            rs2 = big.tile([128, 1], f32)
            nc.vector.reciprocal(out=rs2[:], in_=sd2[:])
            rsg2 = big.tile([128, 1], f32)
            nc.vector.tensor_tensor(out=rsg2[:], in0=rs2[:], in1=g2t[:],
                                    op=OP.mult)
            sh2 = big.tile([128, 1], f32)
            nc.vector.tensor_tensor(out=sh2[:], in0=mn2[:], in1=rsg2[:],
                                    op=OP.mult)
            nc.vector.tensor_tensor(out=sh2[:], in0=b2t[:], in1=sh2[:],
                                    op=OP.subtract)
            nc.vector.tensor_scalar(out=pf[:], in0=pf[:], scalar1=rsg2[:, :1],
                                    scalar2=sh2[:, :1], op0=OP.mult,
                                    op1=OP.add)
            for t in range(4):
                op_ = pa.tile([128, 128], f32, tag="yp")
                nc.tensor.transpose(out=op_[:],
                                    in_=pf[:, t * 128:(t + 1) * 128],
                                    identity=ident[:])
                os_ = sm.tile([128, 128], f32, tag="os")
                nc.vector.tensor_copy(out=os_[:], in_=op_[:])
                nc.sync.dma_start(out=out[t * 128:(t + 1) * 128, :], in_=os_[:])

    nc.compile()
    return nc


def _build_inputs(cores, S, sp_feats, W1, W2, gamma1, beta1, gamma2, beta2):
    KW, NYW, MW, GRW, GW = S["KW"], S["NYW"], S["MW"], S["GRW"], S["GW"]
    W1t = np.ascontiguousarray(
        W1.transpose(1, 0, 2).reshape(C, K3 * C)).astype(BF16)
    base_in = {
        "w1": W1t,
        "g1": gamma1.reshape(C, 1), "b1": beta1.reshape(C, 1),
        "g2": gamma2.reshape(C, 1), "b2": beta2.reshape(C, 1),
        "iot": np.broadcast_to(np.arange(512, dtype=np.float32),
                               (128, 512)).copy(),
    }
    in_maps = []
    for c in range(NCORES):
        L = cores[c]
        m = dict(base_in)
        F = np.zeros((C, NYW), np.float32)
        F[:, L["fcols"]] = sp_feats[L["frows"]].T
        m["f"] = F.astype(BF16)
        w2l = np.zeros((GP, C, C), np.float32)
        p0 = c * GP
        nreal = max(0, min(GP, P - p0))
        w2l[:nreal] = W2[p0:p0 + nreal]
        m["ws"] = np.ascontiguousarray(w2l.sum(0))
        m["w2"] = np.ascontiguousarray(
            w2l.transpose(1, 0, 2).reshape(C, GP * C)).astype(BF16)
        m["g12t"] = _wrap16(L["g12"], MW + GRW)
        m["g34t"] = _wrap16(L["sg"], GW)
        bvw = np.zeros((128, GP), np.float32)
        for lp in range(GP):
            bvw[:, lp] = L["bv"][lp * SLOT:(lp + 1) * SLOT]
        m["bvt"] = bvw
        in_maps.append(m)
    return in_maps


def kernel(**inputs):
    sp_coords = np.asarray(inputs["sp_coords"])
    sp_feats = np.asarray(inputs["sp_feats"], np.float32)
    grid_points = np.asarray(inputs["grid_points"], np.float32)
    W1 = np.asarray(inputs["W1"], np.float32)
    gamma1 = np.asarray(inputs["gamma1"], np.float32)
    beta1 = np.asarray(inputs["beta1"], np.float32)
    W2 = np.asarray(inputs["W2"], np.float32)
    gamma2 = np.asarray(inputs["gamma2"], np.float32)
    beta2 = np.asarray(inputs["beta2"], np.float32)

    Nq, unq_inv, hit_rows, hit_ks, hit_sp = _host_indices(sp_coords,
                                                          grid_points)
    cores, S = _shard(Nq, unq_inv, hit_rows, hit_ks, hit_sp)

    key = tuple(sorted((k, v) for k, v in S.items() if k != "RWS")) \
        + S["RWS"]
    if key not in _cache:
        _cache.clear()
        _cache[key] = _compile(S)
    nc = _cache[key]

    in_maps = _build_inputs(cores, S, sp_feats, W1, W2, gamma1, beta1,
                            gamma2, beta2)

    import os
    from concourse.bass_utils import run_bass_kernel_spmd
    trace = os.environ.get("KERNEL_TRACE", "0") == "1"
    if trace:
        try:
            import ntff_hook
            ntff_hook.install()
        except Exception:
            trace = False
    res = run_bass_kernel_spmd(nc, in_maps, list(range(NCORES)), trace=trace)
    if trace and res.exec_time_ns:
        print("HW exec time: %d ns" % res.exec_time_ns)
    return np.asarray(res.results[0]["out"], np.float32)
